# revision 50
# baseline (speedup 1.0000x reference)
"""Trainium2 Bass kernel for DepthBBoxProcessor (v3).

For each of 4096 bboxes: 7x7 bilinear grid-sample on the depth map of the
box's image, mean over the 49 samples, appended as column 7 of the output.

Key observations exploited:
  * For this input distribution the grid NEVER clips at the image border
    (|cx| <= 0.9532 < 1), and the 7 sample positions per axis are evenly
    spaced with step s = (dim-1)/dim ~ 0.9995 px.  Approximating s = 1
    (anchored at the CENTER sample; measured max rel err 7e-4 vs the
    reference, gate is 2e-2) collapses the separable accumulated weights
    to a closed form:  wt = [1-f, 1, 1, 1, 1, 1, 1, f].
  * Every box needs only an 8x8 pixel patch.  HW indirect DMA gathers ONE
    contiguous window per partition per instruction (multi-index offset
    APs do not work on silicon), so the depth map is repacked on host into
    8-row BANDS, column-major within the band:
        band[img][y0][c][r] = img[y0 + r][c]
    Then the 8x8 patch of a box IS the contiguous 64-float window at
    8-float-unit address  img*2073600 + y0*1920 + x0  -- affine in the
    window origin, no per-box block/parity math, f32-exact (< 2^24).
  * One indirect gather per 128-box group (4 total), each [128 part, 64
    floats] with a [128,1] offset AP -- the exact shape proven on HW.
  * The silicon f32->i32 cast rounds to nearest-even (probed), so
    floor(ph) = cast(ph - 0.4995) with the bias folded into the host
    constants: the whole gather-index chain is 5 DVE ops.  Host staging
    validates coordinate ranges (replacing out-of-range boxes with safe
    coords + exact host fallback), so the device needs no clamps.
  * Weighted mean = sum(patch * (wx (x) wy)) / 49 with 1/49 folded into
    the weight outer product; fused multiply+accumulate per group.
    17 DVE ops total per iteration.

Sharding (8 cores): boxes are sorted by batch id and split into 8 equal
chunks of 512 (G=4 groups of 128, no padding waste).  Each core receives
the IMG_SLOTS=4 consecutive depth images its chunk can reference
(batch-id-aware routing).  Host unpermutes per-core results.  Boxes whose
chunk would span >4 images or that touch the clip region fall back to an
exact host computation (never happens for the reference distribution).
"""

import os
import sys

import numpy as np

if "/opt/trn_rl_repo" not in sys.path:
    sys.path.insert(0, "/opt/trn_rl_repo")

import concourse.bacc as bacc
import concourse.bass as bass
import concourse.mybir as mybir
import concourse.tile as tile
from concourse.bass_utils import run_bass_kernel_spmd

H, W = 1080, 1920
HW = H * W
B = 16
N_CORES = 8
S = 512          # boxes per core (exact split)
G = S // 128     # 4 free-dim groups of 128 boxes
IMG_SLOTS = 4    # consecutive depth images staged per core
NB8 = HW         # 8-float band units per image (1080*1920 window origins)
F32 = mybir.dt.float32
F16 = mybir.dt.float16
I32 = mybir.dt.int32
ALU = mybir.AluOpType
AX = mybir.AxisListType

SX3 = 3.0 * (W - 1) / W   # 3 sample steps in pixels, x
SY3 = 3.0 * (H - 1) / H
# Baked into the host-prescaled columns: the silicon f32->i32 cast rounds to
# nearest-even (probed), so round(ph - 0.4995) == floor(ph) and the floor
# needs no is_gt correction.  (CoreSim's cast truncates instead, so sim
# results shift half the windows by -1px; HW is the correctness gate.)
FLOOR_BIAS = 0.4995
# single averaged delta for the bin-7 weight (per-axis exact values differ
# by 1.2e-3; the resulting weight error is ~6e-4 -- negligible vs the gate)
DELTA_AVG = (SX3 - 3.0 + SY3 - 3.0) / 2.0 + FLOOR_BIAS
# stage depth bands in f16: halves staging + gather bytes and the big DVE
# ops' element time; adds ~5e-4 relative error (f16 mantissa), far under
# the 2e-2 gate.  Validated on silicon by test.py's correctness check.
DEPTH_F16 = True

# box columns in the blob (host pre-scaled so ph = colA + colB per axis):
# [img, x1*959.5 - SX3, y1*539.5 - SY3, x2*959.5, y2*539.5]
BOX_C = 5
# const layout (one row, replicated to 128 partitions):
# [0:2]   zeros (max-with-zero operand)
# [2:2+16G]  wtc[g][a][j] = (j+1) + 3 - S3[a] - FLOOR_BIAS  (wt ramp, tiled
#            per group so the wt build op stays 3-D: [128, G*2, 8])
_C_ZERO, _C_WTC = 0, 2
_C_TOT = _C_WTC + 16 * G
BLOB_W = G * BOX_C + _C_TOT


def _const_row() -> np.ndarray:
    f = np.float32
    iota = np.arange(1, 9, dtype=np.float64)
    wtc = np.concatenate([(iota + 3.0 - SX3), (iota + 3.0 - SY3)])
    row = np.concatenate([
        np.zeros(2, f),
        np.tile(wtc - FLOOR_BIAS, G).astype(f),
    ])
    assert row.shape[0] == _C_TOT
    return np.tile(row[None, :], (128, 1)).astype(f)


def build_nc(n_iters: int = 1, hw_loop: bool = False, unroll: int = 16,
             bufs: int = 3, store_eng: str = "sync",
             wt_pool: bool = False, single_packet: bool = False,
             depth_f16: bool | None = None) -> bass.Bass:
    if depth_f16 is None:
        depth_f16 = DEPTH_F16
    DD = F16 if depth_f16 else F32
    nc = bacc.Bacc()
    # [p, g*5+c] = boxes[g*128+p, c]; [p, G*5:] = per-partition const row
    blob = nc.dram_tensor("blob", [128, BLOB_W], F32, kind="ExternalInput")
    # 8-row bands, column-major: depth[img*HW + y0*W + x0, r] = img[y0+r][x0]
    depth = nc.dram_tensor("depth", [IMG_SLOTS * NB8, 8], DD, kind="ExternalInput")
    avg_out = nc.dram_tensor("avg", [128, G], F32, kind="ExternalOutput")

    with tile.TileContext(nc) as tc:
        with tc.tile_pool(name="p", bufs=(bufs if n_iters > 1 else 1)) as pool:
          def body():
            v = nc.vector
            blob_sb = pool.tile([128, BLOB_W], F32, tag="blob")
            nc.sync.dma_start(out=blob_sb[:], in_=blob[:, :])
            bb = blob_sb[:, 0:G * BOX_C].rearrange("p (g c) -> p g c", g=G)
            cst = blob_sb[:, G * BOX_C:BLOB_W]

            # ---- gather-index chain first (packed x/y in last dim) ----
            ph = pool.tile([128, G, 2], F32, tag="ph")     # first-sample px coords
            v.tensor_tensor(out=ph[:], in0=bb[:, :, 1:3], in1=bb[:, :, 3:5], op=ALU.add)
            # floor(ph + FLOOR_BIAS) via the round-to-nearest cast (see above).
            # Host staging validates coordinate ranges (out-of-range boxes are
            # replaced with safe values + exact host fallback), so no clamps.
            ri = pool.tile([128, G, 2], I32, tag="ri")
            v.tensor_copy(out=ri[:], in_=ph[:])
            xy0 = pool.tile([128, G, 2], F32, tag="xy0")   # (x0, y0) window origin
            v.tensor_copy(out=xy0[:], in_=ri[:])

            r0 = pool.tile([128, G, 1], F32, tag="r0")     # y0*W + x0
            v.scalar_tensor_tensor(out=r0[:], in0=xy0[:, :, 1:2], scalar=float(W),
                                   in1=xy0[:, :, 0:1], op0=ALU.mult, op1=ALU.add)
            idx = pool.tile([128, G, 1], I32, tag="idx")   # img*HW + y0*W + x0
            v.scalar_tensor_tensor(out=idx[:], in0=bb[:, :, 0:1], scalar=float(NB8),
                                   in1=r0[:], op0=ALU.mult, op1=ALU.add)

            # ---- one gather per group: 64-float window == the 8x8 patch ----
            st = pool.tile([128, G, 64], DD, tag="st")
            for gi in range(G):
                gins = nc.gpsimd.indirect_dma_start(
                    out=st[:, gi, :],
                    out_offset=None,
                    in_=depth[:, :],
                    in_offset=bass.IndirectOffsetOnAxis(ap=idx[:, gi, :], axis=0),
                )
                if single_packet:
                    gins.ins.single_packet = True

            # ---- closed-form separable weights (run under the gathers) ----
            wv = nc.gpsimd if wt_pool else v
            fp = pool.tile([128, G, 2], F32, tag="fp")     # ph - xy0 = f - delta
            wv.tensor_tensor(out=fp[:], in0=ph[:], in1=xy0[:], op=ALU.subtract)
            wt = pool.tile([128, G * 2, 8], F32, tag="wt")  # [(g a), j]
            wv.tensor_tensor(out=wt[:],
                             in0=cst[:, _C_WTC:_C_WTC + 16 * G]
                                 .rearrange("p (ga j) -> p ga j", ga=G * 2),
                             in1=fp[:].rearrange("p g a -> p (g a)").unsqueeze(2)
                                 .to_broadcast([128, G * 2, 8]),
                             op=ALU.subtract)
            wv.tensor_scalar(out=wt[:], in0=wt[:], scalar1=1.0, scalar2=0.0,
                             op0=ALU.min, op1=ALU.max)
            # bin 7 = max(f, 0), f = fp + delta (averaged across axes)
            wv.scalar_tensor_tensor(out=wt[:, :, 7:8],
                                    in0=fp[:].rearrange("p g a -> p (g a)").unsqueeze(2),
                                    scalar=float(DELTA_AVG),
                                    in1=cst[:, _C_ZERO:_C_ZERO + 1].unsqueeze(1)
                                        .to_broadcast([128, G * 2, 1]),
                                    op0=ALU.add, op1=ALU.max)
            # weight outer product in band order: wprod[c*8+r] = wx[c]*wy[r]/49
            wprod = pool.tile([128, G, 64], F32, tag="wprod")
            for gi in range(G):
                v.scalar_tensor_tensor(
                    out=wprod[:, gi, :].rearrange("p (c r) -> p c r", c=8),
                    in0=wt[:, 2 * gi, :].unsqueeze(2).to_broadcast([128, 8, 8]),
                    scalar=float(1.0 / 49.0),
                    in1=wt[:, 2 * gi + 1, :].unsqueeze(1).to_broadcast([128, 8, 8]),
                    op0=ALU.mult, op1=ALU.mult)

            # ---- fused multiply + accumulate per group ----
            sm = pool.tile([128, G], F32, tag="sm")
            scratch = pool.tile([128, G, 64], F32, tag="scratch")
            for gi in range(G):
                v.scalar_tensor_tensor(out=scratch[:, gi, :], in0=st[:, gi, :],
                                       scalar=1.0, in1=wprod[:, gi, :],
                                       op0=ALU.mult, op1=ALU.mult,
                                       accum_out=sm[:, gi:gi + 1])
            # store from the otherwise-idle Activation engine (SP keeps blob)
            (nc.scalar if store_eng == "scalar" else nc.sync).dma_start(
                out=avg_out[:, :], in_=sm[:])

          if hw_loop and n_iters > 1:
              assert n_iters % unroll == 0
              with tc.For_i(0, n_iters // unroll):
                  for _u in range(unroll):
                      body()
          else:
              for _it in range(n_iters):
                  body()
    nc.finalize()
    return nc


_NC_CACHE = None


def _get_nc() -> bass.Bass:
    global _NC_CACHE
    if _NC_CACHE is None:
        _NC_CACHE = build_nc()
    return _NC_CACHE


def _host_bilinear(bb: np.ndarray, dm: np.ndarray) -> np.ndarray:
    """Reference-exact fallback for overflow boxes (host, numpy)."""
    f = np.float32
    bids = bb[:, 0].astype(np.int32)
    cx = bb[:, 3] + bb[:, 5] - f(1.0)
    cy = bb[:, 4] + bb[:, 6] - f(1.0)
    offx = np.linspace(-3.0, 3.0, 7).astype(f) / f(W * 0.5)
    offy = np.linspace(-3.0, 3.0, 7).astype(f) / f(H * 0.5)
    gx = np.clip(cx[:, None] + offx[None, :], -1.0, 1.0).astype(f)
    gy = np.clip(cy[:, None] + offy[None, :], -1.0, 1.0).astype(f)
    ix = ((gx + f(1.0)) * f(0.5) * f(W - 1)).astype(f)
    iy = ((gy + f(1.0)) * f(0.5) * f(H - 1)).astype(f)
    x0 = np.floor(ix); y0 = np.floor(iy)
    wx = ix - x0; wy = iy - y0
    x0i = np.clip(x0.astype(np.int32), 0, W - 1); x1i = np.clip(x0i + 1, 0, W - 1)
    y0i = np.clip(y0.astype(np.int32), 0, H - 1); y1i = np.clip(y0i + 1, 0, H - 1)
    d = dm[:, 0]
    bI = bids[:, None, None]
    vv = (d[bI, y0i[:, :, None], x0i[:, None, :]] * (1 - wx)[:, None, :] * (1 - wy)[:, :, None]
          + d[bI, y0i[:, :, None], x1i[:, None, :]] * wx[:, None, :] * (1 - wy)[:, :, None]
          + d[bI, y1i[:, :, None], x0i[:, None, :]] * (1 - wx)[:, None, :] * wy[:, :, None]
          + d[bI, y1i[:, :, None], x1i[:, None, :]] * wx[:, None, :] * wy[:, :, None])
    return vv.mean(axis=(1, 2)).astype(f)


def _make_bands(img: np.ndarray) -> np.ndarray:
    """[H, W] -> [H*W, 8] band layout: out[y*W + x, r] = img[min(y+r, H-1), x].

    Rows below the image edge replicate the last row; they are only ever
    multiplied by exactly-zero weights (y0 <= H-8 after clamping)."""
    padded = np.concatenate([img, np.repeat(img[-1:, :], 7, axis=0)], axis=0)
    v = np.lib.stride_tricks.sliding_window_view(padded, 8, axis=0)  # [H, W, 8]
    out = v[:H].reshape(H * W, 8)
    return np.ascontiguousarray(out.astype(np.float16) if DEPTH_F16 else out)


def make_in_maps(bb: np.ndarray, dm: np.ndarray):
    """Stage per-core inputs.  Returns (in_maps, sels, fallback_idx)."""
    bids = bb[:, 0].astype(np.int32)
    order = np.argsort(bids, kind="stable")
    n = bb.shape[0]
    assert n == N_CORES * S, f"expected {N_CORES * S} boxes, got {n}"
    # boxes in the clip region are approximated wrongly -> host fallback
    cxy = bb[:, 3:5] + bb[:, 5:7] - 1.0
    eps = np.float32(3.2 / (H * 0.5))
    clipbad = (np.abs(cxy) > 1.0 - eps).any(axis=1)
    bands = {}
    in_maps, sels, fallback = [], [], []
    crow = _const_row()
    for c in range(N_CORES):
        sel = order[c * S:(c + 1) * S]
        sels.append(sel)
        bsel = bids[sel]
        lo = int(min(bsel.min(), B - IMG_SLOTS))
        img = bsel - lo
        bad = (img < 0) | (img >= IMG_SLOTS) | clipbad[sel]
        bsub = bb[sel].astype(np.float64)
        boxes_c = np.empty((S, BOX_C), np.float32)
        boxes_c[:, 0] = np.clip(img, 0, IMG_SLOTS - 1).astype(np.float32)
        boxes_c[:, 1] = (bsub[:, 3] * 959.5 - SX3 - FLOOR_BIAS).astype(np.float32)
        boxes_c[:, 2] = (bsub[:, 4] * 539.5 - SY3 - FLOOR_BIAS).astype(np.float32)
        boxes_c[:, 3] = (bsub[:, 5] * 959.5).astype(np.float32)
        boxes_c[:, 4] = (bsub[:, 6] * 539.5).astype(np.float32)
        # replicate the device f32 math to validate window origins; the device
        # has no clamps, so out-of-range boxes get safe coords + host fallback
        phx = boxes_c[:, 1] + boxes_c[:, 3]
        phy = boxes_c[:, 2] + boxes_c[:, 4]
        x0h = np.rint(phx)
        y0h = np.rint(phy)
        bad = bad | (x0h < 0) | (x0h > W - 8) | (y0h < 0) | (y0h > H - 8)
        if bad.any():
            fallback.append(sel[bad])
            boxes_c[bad, 1:5] = np.float32([500.0, 500.0, 0.0, 0.0])
        blob = np.concatenate(
            [boxes_c.reshape(G, 128, BOX_C).transpose(1, 0, 2).reshape(128, G * BOX_C),
             crow], axis=1).astype(np.float32)
        for i in range(lo, lo + IMG_SLOTS):
            if i not in bands:
                bands[i] = _make_bands(dm[i, 0])
        depth_c = np.concatenate([bands[i] for i in range(lo, lo + IMG_SLOTS)], axis=0)
        in_maps.append({"blob": blob, "depth": depth_c})
    fb = np.concatenate(fallback) if fallback else np.empty(0, np.int64)
    return in_maps, sels, fb


def run(inputs: dict, trace: bool = False):
    """Returns (full_output [N,8] f32, BassKernelResults)."""
    bb = np.ascontiguousarray(np.asarray(inputs["bboxes"], dtype=np.float32))
    dm = np.ascontiguousarray(np.asarray(inputs["depth_map"], dtype=np.float32))
    n = bb.shape[0]
    in_maps, sels, fb = make_in_maps(bb, dm)

    nc = _get_nc()
    if os.environ.get("BASS_KERNEL_SIM") == "1":
        from concourse.bass_interp import CoreSim
        res, br = [], None
        for c in range(N_CORES):
            sim = CoreSim(nc)
            for k_, v_ in in_maps[c].items():
                sim.tensor(k_)[:] = v_
            sim.simulate()
            res.append({"avg": np.array(sim.tensor("avg"))})
    else:
        br = run_bass_kernel_spmd(nc, in_maps, list(range(N_CORES)), trace=trace)
        res = br.results

    avg = np.empty((n, 1), np.float32)
    for c in range(N_CORES):
        # device layout is [p, g]; host order within the chunk is g*128+p
        avg[sels[c], 0] = res[c]["avg"].reshape(128, G).T.reshape(-1)
    if len(fb):
        avg[fb, 0] = _host_bilinear(bb[fb], dm)
    return np.concatenate([bb, avg], axis=1), br


def kernel(**inputs) -> np.ndarray:
    out, _ = run(inputs)
    return out
